# revision 17
# baseline (speedup 1.0000x reference)
"""Trainium2 Bass kernel for ragged KeyQueryAttention pooling.

Math (per batch b):
    logits[t] = sum_l (x @ K)[t,l] * (x @ Q)[t,l] = x[t] @ A @ x[t]^T
    with A = (K Q^T + Q K^T)/2  (host-precomputed, [D, D] symmetric)
    att = softmax(logits over valid t)
    out[b]    = sum_t att[t] * x[t, :] + bias        (sum att == 1)

Device strategy (8 NeuronCores, data-parallel over batch):
  - B=64 batches sorted by length (desc), grouped into 8 slots of 8;
    core i takes batch rank 8*j+i for slot j. All cores share one SPMD
    program whose per-slot chunk counts n_j = ceil(max_group_len/128)
    are compiled from the actual lengths (value-specialized; rebuilt per
    call). Rows past each batch's length are masked with -1e30.
  - fp16 on-chip: gpsimd (SWDGE) DMAs cast fp32 HBM -> fp16 SBUF, so
    PE matmuls run at 1 cycle/row (4x the fp32 rate). fp32 is kept for
    logits/softmax accumulation (rel err ~4e-3 vs 2e-2 budget).
  - Per 128-row chunk: PE transpose (fp16) -> xT in PSUM (paired, two
    chunks per PSUM bank); ACT copies the pair to SBUF in one op; PE
    matmul xT.T @ A -> H; one fused DVE scalar_tensor_tensor
    (H * x, row-sum accum) -> logits column. PE transposes run two
    pairs ahead of the H matmuls so neither engine ping-pong stalls.
  - Per slot: additive mask + row max (DVE), global max via gpsimd
    partition_all_reduce (+ negate on gpsimd), ACT exp with accum_out
    giving per-partition Z, then PE weighted-sum matmuls (x chunk
    stationary, att column moving) accumulating acc[128,1] in PSUM.
  - Slots are software-pipelined (chunk phase j, softmax tail j-1,
    weighted sum j-2) so no engine stalls on another's slot tail.
  - Host: out = acc / sum(zrow) + bias, un-permute batches.
"""

import os
import numpy as np

import concourse.bass as bass
import concourse.bacc as bacc
import concourse.tile as tile
from concourse import mybir
from concourse import bass_isa
from concourse.bass_utils import run_bass_kernel_spmd
from concourse.masks import make_identity

B, T, D, L = 64, 8192, 128, 64
NCORES = 8
SLOTS = B // NCORES  # 8 slots per core
F32 = mybir.dt.float32
F16 = mybir.dt.float16

LAST_EXEC_NS = None  # filled when KQA_TRACE=1

_PROG_CACHE = {}


def _build_program(n_list):
    nc = bacc.Bacc()
    ntot = sum(n_list)
    xs = [
        nc.declare_dram_parameter(f"x{j}", [n, 128, D], F32, isOutput=False)
        for j, n in enumerate(n_list)
    ]
    amat = nc.declare_dram_parameter("amat", [D, D], F32, isOutput=False)
    maskp = nc.declare_dram_parameter("mask", [128, ntot], F32, isOutput=False)
    outp = nc.declare_dram_parameter("out", [128, 2 * SLOTS], F32, isOutput=True)

    AF = mybir.ActivationFunctionType
    ALU = mybir.AluOpType

    with tile.TileContext(nc) as tc:
        with (
            tc.tile_pool(name="consts", bufs=1) as consts,
            tc.tile_pool(name="xpool", bufs=5) as xpool,
            tc.tile_pool(name="tpool", bufs=3) as tpool,
            tc.tile_pool(name="jpool", bufs=2) as jpool,
            tc.tile_pool(name="spool", bufs=3) as spool,
            tc.tile_pool(name="psT", bufs=2, space="PSUM") as psT,
            tc.tile_pool(name="psG", bufs=5, space="PSUM") as psG,
            tc.tile_pool(name="psW", bufs=1, space="PSUM") as psW,
        ):
            id16 = consts.tile([128, 128], F16)
            make_identity(nc, id16)
            a_f32 = consts.tile([D, D], F32)
            nc.sync.dma_start(out=a_f32, in_=amat[:, :])
            a_sb = consts.tile([D, D], F16)
            nc.vector.tensor_copy(a_sb, a_f32)
            mask_sb = consts.tile([128, ntot], F32)
            nc.sync.dma_start(out=mask_sb, in_=maskp[:, :])
            out_sb = consts.tile([128, 2 * SLOTS], F32)

            off = [0] * SLOTS
            o = 0
            for j, n in enumerate(n_list):
                off[j] = o
                o += n

            x_sbs = [None] * SLOTS
            p_sbs = [None] * SLOTS
            logits_sbs = [None] * SLOTS

            def load(j):
                n = n_list[j]
                x_sb = xpool.tile([128, n, D], F16, tag="x")
                x_sbs[j] = x_sb
                # SWDGE (gpsimd) DMA casts fp32 DRAM -> fp16 SBUF on the fly.
                h = (n + 1) // 2
                for c0, c1 in ((0, h), (h, n)):
                    if c1 > c0:
                        nc.gpsimd.dma_start(
                            out=x_sb[:, c0:c1, :],
                            in_=xs[j][c0:c1, :, :].rearrange("c t d -> t c d"),
                        )

            def chunk_phase(j):
                n = n_list[j]
                x_sb = x_sbs[j]
                logits = spool.tile([128, n], F32, tag="logits")
                logits_sbs[j] = logits
                pairs = [(c, min(c + 2, n) - c) for c in range(0, n, 2)]
                xT_pairs = [None] * len(pairs)

                def compute(k):
                    c0, w = pairs[k]
                    for i in range(w):
                        c = c0 + i
                        # single-chunk PSUM tile: tile-granular deps mean a
                        # shared multi-chunk tile would serialize H with the
                        # stt reading the neighboring chunk
                        g_ps = psG.tile([128, D], F32, tag="g")
                        nc.tensor.matmul(
                            g_ps,
                            xT_pairs[k][:, i, :],
                            a_sb,
                            start=True,
                            stop=True,
                        )
                        junk = jpool.tile([128, D], F16, tag="junk")
                        # logits[t, c] = sum_d H[t, d] * x[t, d]
                        # (H from PSUM, x from SBUF: one PSUM operand max)
                        nc.vector.scalar_tensor_tensor(
                            out=junk,
                            in0=g_ps,
                            scalar=1.0,
                            in1=x_sb[:, c, :],
                            op0=ALU.mult,
                            op1=ALU.mult,
                            accum_out=logits[:, c : c + 1],
                        )

                for k, (c0, w) in enumerate(pairs):
                    # paired transposes share a PSUM bank: WAW within the PE
                    # is program-order, and the pair copy amortizes ACT's
                    # fixed SBUF-access cost
                    xT_ps = psT.tile([128, 2, 128], F16, tag="xT")
                    for i in range(w):
                        nc.tensor.transpose(
                            xT_ps[:, i, :], x_sb[:, c0 + i, :], id16
                        )
                    xT_sb = tpool.tile([128, 2, 128], F16, tag="xTs")
                    xT_pairs[k] = xT_sb
                    nc.scalar.copy(xT_sb[:, :w, :], xT_ps[:, :w, :])
                    # PE transposes run 2 pairs ahead of the H matmuls
                    if k >= 2:
                        compute(k - 2)
                for k in range(max(0, len(pairs) - 2), len(pairs)):
                    compute(k)
                # ragged mask (additive -1e30 on invalid rows)
                # (gpsimd: SBUF-only ops, keeps DVE free for the stt stream)
                nc.gpsimd.tensor_tensor(
                    logits, logits, mask_sb[:, off[j] : off[j] + n], op=ALU.add
                )
                return logits

            def softmax_tail(j, logits):
                n = n_list[j]
                # global max (whole-tile reduce), negate, broadcast: gpsimd
                maxs = spool.tile([1, 1], F32, tag="maxs")
                nc.gpsimd.tensor_reduce(
                    maxs, logits, axis=mybir.AxisListType.XYZWC, op=ALU.max
                )
                negs = spool.tile([1, 1], F32, tag="negs")
                nc.gpsimd.tensor_scalar_mul(negs, maxs, -1.0)
                negm = spool.tile([128, 1], F32, tag="negm")
                nc.gpsimd.partition_broadcast(negm, negs)
                # P = exp(logits - max) in fp16; zrow = per-partition sum of P
                p_sb = spool.tile([128, n], F16, tag="p")
                p_sbs[j] = p_sb
                nc.scalar.activation(
                    p_sb,
                    logits_sbs[j],
                    AF.Exp,
                    bias=negm,
                    scale=1.0,
                    accum_out=out_sb[:, SLOTS + j : SLOTS + j + 1],
                )

            def wsum(j):
                n = n_list[j]
                x_sb = x_sbs[j]
                p_sb = p_sbs[j]
                acc_ps = psW.tile([128, 1], F32, tag="acc")
                for c in range(n):
                    nc.tensor.matmul(
                        acc_ps,
                        x_sb[:, c, :],
                        p_sb[:, c : c + 1],
                        start=(c == 0),
                        stop=(c == n - 1),
                    )
                nc.scalar.copy(out_sb[:, j : j + 1], acc_ps)

            # software pipeline: chunk phase j || softmax tail j-1 || wsum j-2
            load(0)
            load(1)
            logits_l = [None] * SLOTS
            for j in range(SLOTS):
                logits_l[j] = chunk_phase(j)
                if j >= 1:
                    softmax_tail(j - 1, logits_l[j - 1])
                if j + 2 < SLOTS:
                    load(j + 2)
                if j >= 2:
                    wsum(j - 2)
            softmax_tail(SLOTS - 1, logits_l[SLOTS - 1])
            wsum(SLOTS - 2)
            wsum(SLOTS - 1)
            nc.sync.dma_start(out=outp[:, :], in_=out_sb)
    nc.finalize()
    return nc


def kernel(seq, lengths, key_w, query_w, bias):
    global LAST_EXEC_NS
    seq = np.asarray(seq, dtype=np.float32)
    lengths_np = np.asarray(lengths).astype(np.int64)
    key_w = np.asarray(key_w, dtype=np.float32)
    query_w = np.asarray(query_w, dtype=np.float32)
    bias = np.asarray(bias, dtype=np.float32)

    order = np.argsort(-lengths_np, kind="stable")  # descending length
    n_desc = []
    for j in range(SLOTS):
        grp = order[j * NCORES : (j + 1) * NCORES]
        n_desc.append(max(1, int(-(-int(lengths_np[grp].max()) // 128))))
    # process smallest slots first so the pipeline prologue is cheap
    slot_perm = sorted(range(SLOTS), key=lambda j: n_desc[j])
    n_list = [n_desc[j] for j in slot_perm]
    key = tuple(n_list)
    if key not in _PROG_CACHE:
        _PROG_CACHE[key] = _build_program(n_list)
    nc = _PROG_CACHE[key]

    amat_np = (key_w @ query_w.T + query_w @ key_w.T) * 0.5  # [D, D] symmetric
    in_maps = []
    for i in range(NCORES):
        m = {"amat": amat_np}
        mask_cols = []
        for js, jd in enumerate(slot_perm):
            n = n_list[js]
            b = int(order[jd * NCORES + i])
            m[f"x{js}"] = seq[b, : n * 128, :].reshape(n, 128, D)
            lb = int(lengths_np[b])
            col = np.where(np.arange(n * 128) < lb, 0.0, -1e30).astype(np.float32)
            mask_cols.append(col.reshape(n, 128).T)  # [128, n]
        m["mask"] = np.ascontiguousarray(np.concatenate(mask_cols, axis=1))
        in_maps.append(m)

    trace = os.environ.get("KQA_TRACE") == "1"
    res = run_bass_kernel_spmd(
        nc, in_maps, core_ids=list(range(NCORES)), trace=trace
    )
    LAST_EXEC_NS = res.exec_time_ns

    out = np.empty((B, D), dtype=np.float32)
    for i in range(NCORES):
        r = res.results[i]["out"]  # [128, 2*SLOTS]
        for js, jd in enumerate(slot_perm):
            b = int(order[jd * NCORES + i])
            acc = r[:, js]
            z = r[:, SLOTS + js].sum(dtype=np.float64)
            out[b] = (acc / z).astype(np.float32) + bias
    return out


# revision 18
# speedup vs baseline: 1.8336x; 1.8336x over previous
"""Trainium2 Bass kernel for ragged KeyQueryAttention pooling.

Math (per batch b):
    logits[t] = sum_l (x @ K)[t,l] * (x @ Q)[t,l] = x[t] @ A @ x[t]^T
    with A = (K Q^T + Q K^T)/2  (host-precomputed, [D, D] symmetric)
    att = softmax(logits over valid t)
    out[b]    = sum_t att[t] * x[t, :] + bias        (sum att == 1)

Device strategy (8 NeuronCores, data-parallel over batch):
  - B=64 batches sorted by length (desc), grouped into 8 slots of 8;
    core i takes batch rank 8*j+i for slot j. All cores share one SPMD
    program whose per-slot chunk counts n_j = ceil(max_group_len/128)
    are compiled from the actual lengths (value-specialized; rebuilt per
    call). Rows past each batch's length are masked with -1e30.
  - fp16 on-chip: gpsimd (SWDGE) DMAs cast fp32 HBM -> fp16 SBUF, so
    PE matmuls run at 1 cycle/row (4x the fp32 rate). fp32 is kept for
    logits/softmax accumulation (rel err ~4e-3 vs 2e-2 budget).
  - Per 128-row chunk: PE transpose (fp16) -> xT in PSUM (paired, two
    chunks per PSUM bank); ACT copies the pair to SBUF in one op; PE
    matmul xT.T @ A -> H; one fused DVE scalar_tensor_tensor
    (H * x, row-sum accum) -> logits column. PE transposes run two
    pairs ahead of the H matmuls so neither engine ping-pong stalls.
  - Per slot: additive mask + row max (DVE), global max via gpsimd
    partition_all_reduce (+ negate on gpsimd), ACT exp with accum_out
    giving per-partition Z, then PE weighted-sum matmuls (x chunk
    stationary, att column moving) accumulating acc[128,1] in PSUM.
  - Slots are software-pipelined (chunk phase j, softmax tail j-1,
    weighted sum j-2) so no engine stalls on another's slot tail.
  - Host: out = acc / sum(zrow) + bias, un-permute batches.
"""

import os
import numpy as np

import concourse.bass as bass
import concourse.bacc as bacc
import concourse.tile as tile
from concourse import mybir
from concourse import bass_isa
from concourse.bass_utils import run_bass_kernel_spmd
from concourse.masks import make_identity

B, T, D, L = 64, 8192, 128, 64
NCORES = 8
SLOTS = B // NCORES  # 8 slots per core
F32 = mybir.dt.float32
F16 = mybir.dt.float16

LAST_EXEC_NS = None  # filled when KQA_TRACE=1

_PROG_CACHE = {}


def _build_program(n_list):
    nc = bacc.Bacc()
    ntot = sum(n_list)
    xs = [
        nc.declare_dram_parameter(f"x{j}", [n, 128, D], F32, isOutput=False)
        for j, n in enumerate(n_list)
    ]
    amat = nc.declare_dram_parameter("amat", [D, D], F32, isOutput=False)
    maskp = nc.declare_dram_parameter("mask", [128, ntot], F32, isOutput=False)
    outp = nc.declare_dram_parameter("out", [128, 2 * SLOTS], F32, isOutput=True)

    AF = mybir.ActivationFunctionType
    ALU = mybir.AluOpType

    with tile.TileContext(nc) as tc:
        with (
            tc.tile_pool(name="consts", bufs=1) as consts,
            tc.tile_pool(name="xpool", bufs=5) as xpool,
            tc.tile_pool(name="tpool", bufs=4) as tpool,
            tc.tile_pool(name="jpool", bufs=2) as jpool,
            tc.tile_pool(name="spool", bufs=3) as spool,
            tc.tile_pool(name="psT", bufs=3, space="PSUM") as psT,
            tc.tile_pool(name="psG", bufs=4, space="PSUM") as psG,
            tc.tile_pool(name="psX", bufs=1, space="PSUM") as psX,
        ):
            id16 = consts.tile([128, 128], F16)
            make_identity(nc, id16)
            id32 = consts.tile([128, 128], F32)
            make_identity(nc, id32)
            neg_row = consts.tile([1, 128], F32)
            nc.vector.memset(neg_row, -1.0)
            a_f32 = consts.tile([D, D], F32)
            nc.sync.dma_start(out=a_f32, in_=amat[:, :])
            a_sb = consts.tile([D, D], F16)
            nc.vector.tensor_copy(a_sb, a_f32)
            mask_sb = consts.tile([128, ntot], F32)
            nc.sync.dma_start(out=mask_sb, in_=maskp[:, :])
            out_sb = consts.tile([128, 2 * SLOTS], F32)
            # one PSUM bank shared by the max-chain and the wsum accumulator
            # (disjoint columns; serial per-slot use)
            amx_tile = psX.tile([128, 512], F32, tag="amx")

            off = [0] * SLOTS
            o = 0
            for j, n in enumerate(n_list):
                off[j] = o
                o += n

            x_sbs = [None] * SLOTS
            p_sbs = [None] * SLOTS
            logits_sbs = [None] * SLOTS

            def load(j):
                n = n_list[j]
                x_sb = xpool.tile([128, n, D], F16, tag="x")
                x_sbs[j] = x_sb
                # SWDGE (gpsimd) DMA casts fp32 DRAM -> fp16 SBUF on the fly.
                h = (n + 1) // 2
                for c0, c1 in ((0, h), (h, n)):
                    if c1 > c0:
                        nc.gpsimd.dma_start(
                            out=x_sb[:, c0:c1, :],
                            in_=xs[j][c0:c1, :, :].rearrange("c t d -> t c d"),
                        )

            def chunk_phase(j):
                n = n_list[j]
                x_sb = x_sbs[j]
                logits = spool.tile([128, n], F32, tag="logits")
                logits_sbs[j] = logits
                pairs = [(c, min(c + 2, n) - c) for c in range(0, n, 2)]
                xT_pairs = [None] * len(pairs)

                def compute(k):
                    c0, w = pairs[k]
                    for i in range(w):
                        c = c0 + i
                        # single-chunk PSUM tile: tile-granular deps mean a
                        # shared multi-chunk tile would serialize H with the
                        # stt reading the neighboring chunk
                        g_ps = psG.tile([128, D], F32, tag="g")
                        nc.tensor.matmul(
                            g_ps,
                            xT_pairs[k][:, i, :],
                            a_sb,
                            start=True,
                            stop=True,
                        )
                        junk = jpool.tile([128, D], F16, tag="junk")
                        # logits[t, c] = sum_d H[t, d] * x[t, d]
                        # (H from PSUM, x from SBUF: one PSUM operand max)
                        nc.vector.scalar_tensor_tensor(
                            out=junk,
                            in0=g_ps,
                            scalar=1.0,
                            in1=x_sb[:, c, :],
                            op0=ALU.mult,
                            op1=ALU.mult,
                            accum_out=logits[:, c : c + 1],
                        )

                for k, (c0, w) in enumerate(pairs):
                    # paired transposes share a PSUM bank: WAW within the PE
                    # is program-order, and the pair copy amortizes ACT's
                    # fixed SBUF-access cost
                    xT_ps = psT.tile([128, 2, 128], F16, tag="xT")
                    for i in range(w):
                        nc.tensor.transpose(
                            xT_ps[:, i, :], x_sb[:, c0 + i, :], id16
                        )
                    xT_sb = tpool.tile([128, 2, 128], F16, tag="xTs")
                    xT_pairs[k] = xT_sb
                    nc.scalar.copy(xT_sb[:, :w, :], xT_ps[:, :w, :])
                    # PE transposes run 2 pairs ahead of the H matmuls
                    if k >= 2:
                        compute(k - 2)
                for k in range(max(0, len(pairs) - 2), len(pairs)):
                    compute(k)
                # ragged mask (additive -1e30 on invalid rows) + row maxes
                nc.vector.tensor_tensor(
                    logits, logits, mask_sb[:, off[j] : off[j] + n], op=ALU.add
                )
                rowmax = spool.tile([128, 1], F32, tag="rmax")
                nc.vector.tensor_reduce(
                    rowmax, logits, axis=mybir.AxisListType.X, op=ALU.max
                )
                return rowmax

            def softmax_tail(j, rowmax):
                n = n_list[j]
                # global max: PE transpose of rowmax + DVE reduce, then
                # broadcast -max to all partitions via (-1s).T @ max
                amx = amx_tile
                nc.tensor.transpose(amx[0:1, 0:128], rowmax, id32)
                maxs = spool.tile([1, 1], F32, tag="maxs")
                nc.vector.tensor_reduce(
                    maxs, amx[0:1, 0:128], axis=mybir.AxisListType.X, op=ALU.max
                )
                nc.tensor.matmul(
                    amx[:, 200:201], neg_row, maxs, start=True, stop=True
                )
                negm = spool.tile([128, 1], F32, tag="negm")
                nc.vector.tensor_copy(negm, amx[:, 200:201])
                # P = exp(logits - max) in fp16; zrow = per-partition sum of P
                p_sb = spool.tile([128, n], F16, tag="p")
                p_sbs[j] = p_sb
                nc.scalar.activation(
                    p_sb,
                    logits_sbs[j],
                    AF.Exp,
                    bias=negm,
                    scale=1.0,
                    accum_out=out_sb[:, SLOTS + j : SLOTS + j + 1],
                )

            def wsum(j):
                n = n_list[j]
                x_sb = x_sbs[j]
                p_sb = p_sbs[j]
                acc_ps = amx_tile[:, 300:301]
                for c in range(n):
                    nc.tensor.matmul(
                        acc_ps,
                        x_sb[:, c, :],
                        p_sb[:, c : c + 1],
                        start=(c == 0),
                        stop=(c == n - 1),
                    )
                nc.scalar.copy(out_sb[:, j : j + 1], acc_ps)

            # software pipeline: chunk phase j || softmax tail j-1 || wsum j-2
            load(0)
            load(1)
            rowmaxes = [None] * SLOTS
            for j in range(SLOTS):
                rowmaxes[j] = chunk_phase(j)
                if j >= 1:
                    softmax_tail(j - 1, rowmaxes[j - 1])
                if j + 2 < SLOTS:
                    load(j + 2)
                if j >= 2:
                    wsum(j - 2)
            softmax_tail(SLOTS - 1, rowmaxes[SLOTS - 1])
            wsum(SLOTS - 2)
            wsum(SLOTS - 1)
            nc.sync.dma_start(out=outp[:, :], in_=out_sb)
    nc.finalize()
    return nc


def kernel(seq, lengths, key_w, query_w, bias):
    global LAST_EXEC_NS
    seq = np.asarray(seq, dtype=np.float32)
    lengths_np = np.asarray(lengths).astype(np.int64)
    key_w = np.asarray(key_w, dtype=np.float32)
    query_w = np.asarray(query_w, dtype=np.float32)
    bias = np.asarray(bias, dtype=np.float32)

    order = np.argsort(-lengths_np, kind="stable")  # descending length
    n_desc = []
    for j in range(SLOTS):
        grp = order[j * NCORES : (j + 1) * NCORES]
        n_desc.append(max(1, int(-(-int(lengths_np[grp].max()) // 128))))
    # process smallest slots first so the pipeline prologue is cheap
    slot_perm = sorted(range(SLOTS), key=lambda j: n_desc[j])
    n_list = [n_desc[j] for j in slot_perm]
    key = tuple(n_list)
    if key not in _PROG_CACHE:
        _PROG_CACHE[key] = _build_program(n_list)
    nc = _PROG_CACHE[key]

    amat_np = (key_w @ query_w.T + query_w @ key_w.T) * 0.5  # [D, D] symmetric
    in_maps = []
    for i in range(NCORES):
        m = {"amat": amat_np}
        mask_cols = []
        for js, jd in enumerate(slot_perm):
            n = n_list[js]
            b = int(order[jd * NCORES + i])
            m[f"x{js}"] = seq[b, : n * 128, :].reshape(n, 128, D)
            lb = int(lengths_np[b])
            col = np.where(np.arange(n * 128) < lb, 0.0, -1e30).astype(np.float32)
            mask_cols.append(col.reshape(n, 128).T)  # [128, n]
        m["mask"] = np.ascontiguousarray(np.concatenate(mask_cols, axis=1))
        in_maps.append(m)

    trace = os.environ.get("KQA_TRACE") == "1"
    res = run_bass_kernel_spmd(
        nc, in_maps, core_ids=list(range(NCORES)), trace=trace
    )
    LAST_EXEC_NS = res.exec_time_ns

    out = np.empty((B, D), dtype=np.float32)
    for i in range(NCORES):
        r = res.results[i]["out"]  # [128, 2*SLOTS]
        for js, jd in enumerate(slot_perm):
            b = int(order[jd * NCORES + i])
            acc = r[:, js]
            z = r[:, SLOTS + js].sum(dtype=np.float64)
            out[b] = (acc / z).astype(np.float32) + bias
    return out
